# revision 9
# baseline (speedup 1.0000x reference)
"""Trainium2 Bass kernel for CifNet conv-QKV self-attention.

Sharding: 8 cores = 4 (batch) x 2 (head-groups of 4 heads).
Each core computes, for its batch sample b and head-group g:
  - q/k/v = conv3x3(x, w{q,k,v}[g*256:(g+1)*256])   (256 out-channels = 4 heads)
  - per-head attention over hw=2304 positions (softmax without max-subtraction,
    denominator fused into the AV matmul via an appended ones-column on V^T)
  - partial o-conv: conv3x3(attn_out, wo[:, g*256:(g+1)*256])  -> [256, 2304] fp32
Host sums the two head-group partials per batch sample.

Convs are expressed as 9 shifted matmuls (one per tap) accumulating in PSUM,
with the input pre-padded to [C, 50, 50] on the host. All matmuls run in bf16
with fp32 PSUM accumulation (measured end-to-end rel-l2 error ~5.5e-3).
"""

from contextlib import ExitStack

import numpy as np
import ml_dtypes

# problem shape (hardcoded per contract)
B, C, H, W = 4, 256, 48, 48
HW = H * W              # 2304
NCORES = 8
RT = 8                  # output rows per spatial tile
NT = RT * W             # 384 columns per matmul
NROW = H // RT          # 6 spatial tiles
NKJ = HW // 128         # 18 key tiles
KJG = 3                 # kj tiles per exp group
NGRP = NKJ // KJG       # 6 groups

_cached = None


def _build():
    """Build and compile the per-core SPMD Bass program (cached)."""
    global _cached
    if _cached is not None:
        return _cached

    import concourse.bass as bass  # noqa: F401
    import concourse.tile as tile
    from concourse import bacc, mybir
    from concourse.masks import make_identity

    BF = mybir.dt.bfloat16
    F32 = mybir.dt.float32
    EXP = mybir.ActivationFunctionType.Exp

    nc = bacc.Bacc("TRN2", target_bir_lowering=False, debug=False)
    x_d = nc.dram_tensor("xpad", [2, 128, 50, 50], BF, kind="ExternalInput").ap()
    wqkv_d = nc.dram_tensor("wqkv", [3, 9, 2, 128, 256], BF, kind="ExternalInput").ap()
    wo_d = nc.dram_tensor("wo", [9, 2, 128, 256], BF, kind="ExternalInput").ap()
    out_d = nc.dram_tensor("out", [2, 128, HW], F32, kind="ExternalOutput").ap()

    with tile.TileContext(nc) as tc, ExitStack() as ctx:
        konst = ctx.enter_context(tc.tile_pool(name="konst", bufs=1))
        # identity for PE transpose; duplicated at base partitions 0 and 64 so
        # the transpose input/identity share a base partition
        ident = konst.tile([128, 64], BF, name="ident")
        make_identity(nc, ident[0:64, :])
        nc.sync.dma_start(ident[64:128, :], ident[0:64, :])

        x_sb = konst.tile([128, 2, 50, 50], BF, name="x_sb")
        wq_sb = konst.tile([128, 9, 2, 256], BF, name="wq_sb")
        wk_sb = konst.tile([128, 9, 2, 256], BF, name="wk_sb")
        wv_sb = konst.tile([128, 9, 2, 256], BF, name="wv_sb")
        wo_sb = konst.tile([128, 9, 2, 256], BF, name="wo_sb")
        q_sb = [konst.tile([128, HW], BF, name=f"q_sb{m}") for m in range(2)]
        k_sb = [konst.tile([128, HW], BF, name=f"k_sb{m}") for m in range(2)]
        v_sb = [konst.tile([128, HW], BF, name=f"v_sb{m}") for m in range(2)]
        # V^T per head: [kj within tile, kj tile, 65]; col 64 holds ones so the
        # AV matmul also produces the softmax denominator in psum row 64.
        vt_sb = [konst.tile([128, NKJ, 65], BF, name=f"vt_sb{h}") for h in range(4)]
        opad = [konst.tile([128, 50, 50], BF, name=f"opad{g}") for g in range(2)]

        # input DMAs
        for kg in range(2):
            nc.sync.dma_start(x_sb[:, kg], x_d[kg])
        for a, w_sb in enumerate((wq_sb, wk_sb, wv_sb)):
            for t in range(9):
                nc.sync.dma_start(w_sb[:, t], wqkv_d[a, t].rearrange("g p o -> p g o"))
        for t in range(9):
            nc.sync.dma_start(wo_sb[:, t], wo_d[t].rearrange("g p o -> p g o"))

        for h in range(4):
            nc.gpsimd.memset(vt_sb[h][:], 1.0)
        for g in range(2):
            nc.gpsimd.memset(opad[g][:], 0.0)

        # ---------------- phase 1: qkv convs + v transpose ----------------
        with tc.tile_pool(name="cpsum", bufs=6, space="PSUM") as cpsum, \
             tc.tile_pool(name="tpsum", bufs=2, space="PSUM") as tpsum:
            for m in range(2):
                # v first so its transposes overlap the q/k convs
                for w_sb, dst in ((wv_sb, v_sb[m]), (wq_sb, q_sb[m]), (wk_sb, k_sb[m])):
                    ps = [cpsum.tile([128, NT], F32, tag="cps", name="cps") for _ in range(NROW)]
                    first = True
                    for kg in range(2):
                        for t in range(9):
                            ky, kx = t // 3, t % 3
                            lhsT = w_sb[:, t, kg, m * 128:(m + 1) * 128]
                            last = (kg == 1 and t == 8)
                            for r in range(NROW):
                                rhs = x_sb[:, kg, r * RT + ky: r * RT + ky + RT, kx: kx + W]
                                nc.tensor.matmul(ps[r][:], lhsT, rhs, start=first, stop=last)
                            first = False
                    for r in range(NROW):
                        nc.vector.tensor_copy(dst[:, r * NT:(r + 1) * NT], ps[r][:])
                    if dst is v_sb[m]:
                        for hh in range(2):
                            h = 2 * m + hh
                            for kt in range(NKJ):
                                pt = tpsum.tile([128, 64], BF, tag="tps", name="tps")
                                nc.tensor.transpose(
                                    pt[:],
                                    v_sb[m][64 * hh:64 * hh + 64, kt * 128:(kt + 1) * 128],
                                    ident[64 * hh:64 * hh + 64, :],
                                )
                                nc.vector.tensor_copy(vt_sb[h][:, kt, 0:64], pt[:])

        # ---------------- phase 2: attention ----------------
        with tc.tile_pool(name="spsum", bufs=1, space="PSUM") as spsum, \
             tc.tile_pool(name="apsum", bufs=2, space="PSUM") as apsum, \
             tc.tile_pool(name="esb", bufs=3) as esb, \
             tc.tile_pool(name="nsb", bufs=2) as nsb:
            for m in range(2):          # head-pair == m-tile == o-conv kgroup
                for qi in range(NROW):
                    qsl = slice(qi * NT, (qi + 1) * NT)
                    av = [apsum.tile([128, NT], F32, tag="avps", name="avps") for _ in range(2)]
                    for grp in range(NGRP):
                        sp = spsum.tile([128, 2, KJG, 512], F32, tag="sps", name="sps")
                        for hh in range(2):
                            for j in range(KJG):
                                kjt = grp * KJG + j
                                lhsT = k_sb[m][64 * hh:64 * hh + 64, kjt * 128:(kjt + 1) * 128]
                                rhs = q_sb[m][64 * hh:64 * hh + 64, qsl]
                                nc.tensor.matmul(
                                    sp[:, hh, j, 0:NT], lhsT, rhs,
                                    start=True, stop=True,
                                    tile_position=(64 * hh, 0),
                                )
                        et = esb.tile([128, 2, KJG, NT], BF, tag="et", name="et")
                        nc.scalar.activation(et[:], sp[:, :, :, 0:NT], EXP, scale=0.125)
                        for hh in range(2):
                            h = 2 * m + hh
                            for j in range(KJG):
                                kjt = grp * KJG + j
                                lhsT = vt_sb[h][:, kjt, 0:65]
                                nc.tensor.matmul(
                                    av[hh][0:65, :], lhsT, et[:, hh, j, :],
                                    start=(kjt == 0), stop=(kjt == NKJ - 1),
                                )
                    for hh in range(2):
                        # den sits in psum row 64; DVE copies keep the
                        # partition base, DMA remaps 64 -> 0
                        dn = nsb.tile([128, NT], F32, tag="dn", name="dn")
                        nc.vector.tensor_copy(dn[64:65, :], av[hh][64:65, :])
                        nc.sync.dma_start(dn[0:1, :], dn[64:65, :])
                        rc = nsb.tile([1, NT], F32, tag="rc", name="rc")
                        nc.vector.reciprocal(rc[:], dn[0:1, :])
                        rb = nsb.tile([64, NT], F32, tag="rb", name="rb")
                        nc.gpsimd.partition_broadcast(rb[:], rc[:])
                        tmp = nsb.tile([64, NT], BF, tag="tmp", name="tmp")
                        nc.vector.tensor_mul(tmp[:], av[hh][0:64, :], rb[:])
                        dst = opad[m][64 * hh:64 * hh + 64, qi * RT + 1: qi * RT + RT + 1, 1:49]
                        nc.sync.dma_start(dst, tmp[:].rearrange("p (r c) -> p r c", c=W))

        # ---------------- phase 3: o-conv partial ----------------
        with tc.tile_pool(name="opsum", bufs=6, space="PSUM") as opsum, \
             tc.tile_pool(name="osb", bufs=3) as osb:
            for mo in range(2):
                ps = [opsum.tile([128, NT], F32, tag="ops", name="ops") for _ in range(NROW)]
                first = True
                for kg in range(2):
                    for t in range(9):
                        ky, kx = t // 3, t % 3
                        lhsT = wo_sb[:, t, kg, mo * 128:(mo + 1) * 128]
                        last = (kg == 1 and t == 8)
                        for r in range(NROW):
                            rhs = opad[kg][:, r * RT + ky: r * RT + ky + RT, kx: kx + W]
                            nc.tensor.matmul(ps[r][:], lhsT, rhs, start=first, stop=last)
                        first = False
                for r in range(NROW):
                    ot = osb.tile([128, NT], F32, tag="osb", name="osb")
                    nc.vector.tensor_copy(ot[:], ps[r][:])
                    nc.sync.dma_start(out_d[mo, :, r * NT:(r + 1) * NT], ot[:])

    nc.compile()
    _cached = nc
    return nc


def make_in_maps(hidden_states, wq, wk, wv, wo):
    """Shard + pre-transform full inputs into 8 per-core input dicts."""
    bf = ml_dtypes.bfloat16
    hidden_states = np.asarray(hidden_states, np.float32)
    in_maps = []
    for core in range(NCORES):
        b, g = core // 2, core % 2
        xp = np.zeros((C, 50, 50), np.float32)
        xp[:, 1:49, 1:49] = hidden_states[b]
        xpad = np.ascontiguousarray(xp.reshape(2, 128, 50, 50)).astype(bf)
        wstk = np.stack(
            [
                np.asarray(w, np.float32)[g * 256:(g + 1) * 256]
                .transpose(2, 3, 1, 0)
                .reshape(9, 2, 128, 256)
                for w in (wq, wk, wv)
            ]
        ).astype(bf)
        wog = (
            np.asarray(wo, np.float32)[:, g * 256:(g + 1) * 256]
            .transpose(2, 3, 1, 0)
            .reshape(9, 2, 128, 256)
            .astype(bf)
        )
        in_maps.append({"xpad": xpad, "wqkv": wstk, "wo": wog})
    return in_maps


def combine_outputs(per_core_outs):
    """Sum the two head-group partials per batch sample."""
    out = np.empty((B, C, H, W), np.float32)
    for b in range(B):
        acc = per_core_outs[2 * b].reshape(C, HW).astype(np.float32) + \
              per_core_outs[2 * b + 1].reshape(C, HW).astype(np.float32)
        out[b] = acc.reshape(C, H, W)
    return out


def kernel(hidden_states, wq, wk, wv, wo):
    from concourse.bass_utils import run_bass_kernel_spmd

    nc = _build()
    in_maps = make_in_maps(hidden_states, wq, wk, wv, wo)
    res = run_bass_kernel_spmd(nc, in_maps, core_ids=list(range(NCORES)))
    return combine_outputs([r["out"] for r in res.results])


# revision 12
# speedup vs baseline: 1.1360x; 1.1360x over previous
"""Trainium2 Bass kernel for CifNet conv-QKV self-attention.

Sharding: 8 cores = 4 (batch) x 2 (head-groups of 4 heads).
Each core computes, for its batch sample b and head-group g:
  - q/k/v = conv3x3(x, w{q,k,v}[g*256:(g+1)*256])   (256 out-channels = 4 heads)
  - per-head attention over hw=2304 positions (softmax without max-subtraction,
    denominator fused into the AV matmul via an appended ones-column on V^T)
  - partial o-conv: conv3x3(attn_out, wo[:, g*256:(g+1)*256])  -> [256, 2304] fp32
Host sums the two head-group partials per batch sample.

Convs are expressed as 9 shifted matmuls (one per tap) accumulating in PSUM,
with the input pre-padded to [C, 50, 50] on the host. All matmuls run in bf16
with fp32 PSUM accumulation (measured end-to-end rel-l2 error ~5.5e-3).
"""

from contextlib import ExitStack

import numpy as np
import ml_dtypes

# problem shape (hardcoded per contract)
B, C, H, W = 4, 256, 48, 48
HW = H * W              # 2304
NCORES = 8
RT = 8                  # output rows per spatial tile
NT = RT * W             # 384 columns per matmul
NROW = H // RT          # 6 spatial tiles
NKJ = HW // 128         # 18 key tiles
KJG = 3                 # kj tiles per exp group
NGRP = NKJ // KJG       # 6 groups

_cached = None


def _build():
    """Build and compile the per-core SPMD Bass program (cached)."""
    global _cached
    if _cached is not None:
        return _cached

    import concourse.bass as bass  # noqa: F401
    import concourse.tile as tile
    from concourse import bacc, mybir
    from concourse.masks import make_identity

    BF = mybir.dt.bfloat16
    F32 = mybir.dt.float32
    EXP = mybir.ActivationFunctionType.Exp

    nc = bacc.Bacc("TRN2", target_bir_lowering=False, debug=False)
    x_d = nc.dram_tensor("xpad", [2, 128, 50, 50], BF, kind="ExternalInput").ap()
    wqkv_d = nc.dram_tensor("wqkv", [3, 9, 2, 128, 256], BF, kind="ExternalInput").ap()
    wo_d = nc.dram_tensor("wo", [9, 2, 128, 256], BF, kind="ExternalInput").ap()
    out_d = nc.dram_tensor("out", [2, 128, HW], F32, kind="ExternalOutput").ap()

    with tile.TileContext(nc) as tc, ExitStack() as ctx:
        konst = ctx.enter_context(tc.tile_pool(name="konst", bufs=1))
        # identity for PE transpose; duplicated at base partitions 0 and 64 so
        # the transpose input/identity share a base partition
        ident = konst.tile([128, 64], BF, name="ident")
        make_identity(nc, ident[0:64, :])
        nc.sync.dma_start(ident[64:128, :], ident[0:64, :])

        x_sb = konst.tile([128, 2, 50, 50], BF, name="x_sb")
        wq_sb = konst.tile([128, 9, 2, 256], BF, name="wq_sb")
        wk_sb = konst.tile([128, 9, 2, 256], BF, name="wk_sb")
        wv_sb = konst.tile([128, 9, 2, 256], BF, name="wv_sb")
        wo_sb = konst.tile([128, 9, 2, 256], BF, name="wo_sb")
        q_sb = [konst.tile([128, HW], BF, name=f"q_sb{m}") for m in range(2)]
        k_sb = [konst.tile([128, HW], BF, name=f"k_sb{m}") for m in range(2)]
        v_sb = [konst.tile([128, HW], BF, name=f"v_sb{m}") for m in range(2)]
        # V^T per head: [kj within tile, kj tile, 65]; col 64 holds ones so the
        # AV matmul also produces the softmax denominator in psum row 64.
        vt_sb = [konst.tile([128, NKJ, 65], BF, name=f"vt_sb{h}") for h in range(4)]
        opad = [konst.tile([128, 50, 50], BF, name=f"opad{g}") for g in range(2)]

        # input DMAs
        for kg in range(2):
            nc.sync.dma_start(x_sb[:, kg], x_d[kg])
        for a, w_sb in enumerate((wq_sb, wk_sb, wv_sb)):
            for t in range(9):
                nc.sync.dma_start(w_sb[:, t], wqkv_d[a, t].rearrange("g p o -> p g o"))
        for t in range(9):
            nc.sync.dma_start(wo_sb[:, t], wo_d[t].rearrange("g p o -> p g o"))

        for h in range(4):
            nc.gpsimd.memset(vt_sb[h][:], 1.0)
        for g in range(2):
            nc.gpsimd.memset(opad[g][:], 0.0)

        # warm the ACT exp table during the DMA phase (one-time ~2.7us load)
        wrm = konst.tile([1, 8], F32, name="wrm")
        nc.gpsimd.memset(wrm[:], 0.0)
        nc.scalar.activation(wrm[:], wrm[:], EXP, scale=0.125)

        # ---------------- phase 1: qkv convs + v transpose ----------------
        with tc.tile_pool(name="cpsum", bufs=6, space="PSUM") as cpsum, \
             tc.tile_pool(name="tpsum", bufs=2, space="PSUM") as tpsum:
            for m in range(2):
                # v first so its transposes overlap the q/k convs
                for w_sb, dst in ((wv_sb, v_sb[m]), (wq_sb, q_sb[m]), (wk_sb, k_sb[m])):
                    ps = [cpsum.tile([128, NT], F32, tag="cps", name="cps") for _ in range(NROW)]
                    first = True
                    for kg in range(2):
                        for t in range(9):
                            ky, kx = t // 3, t % 3
                            lhsT = w_sb[:, t, kg, m * 128:(m + 1) * 128]
                            last = (kg == 1 and t == 8)
                            for r in range(NROW):
                                rhs = x_sb[:, kg, r * RT + ky: r * RT + ky + RT, kx: kx + W]
                                nc.tensor.matmul(ps[r][:], lhsT, rhs, start=first, stop=last)
                            first = False
                    for r in range(NROW):
                        nc.vector.tensor_copy(dst[:, r * NT:(r + 1) * NT], ps[r][:])
                    if dst is v_sb[m]:
                        for hh in range(2):
                            h = 2 * m + hh
                            for kt in range(NKJ):
                                pt = tpsum.tile([128, 64], BF, tag="tps", name="tps")
                                nc.tensor.transpose(
                                    pt[:],
                                    v_sb[m][64 * hh:64 * hh + 64, kt * 128:(kt + 1) * 128],
                                    ident[64 * hh:64 * hh + 64, :],
                                )
                                nc.vector.tensor_copy(vt_sb[h][:, kt, 0:64], pt[:])

        # ---------------- phase 2: attention ----------------
        # Pipeline: per (group, head) -> 3 score MMs into a 3-bank psum tile
        # (bufs=2 so the next group's scores overlap this group's exp), one
        # psum-direct Exp into bf16, 3 AV MMs accumulating [v|ones]^T @ E.
        # The AV psum is evicted to SBUF immediately so its bank recycles,
        # and the normalize chain runs entirely from SBUF off-critical-path.
        with tc.tile_pool(name="spsum", bufs=2, space="PSUM") as spsum, \
             tc.tile_pool(name="apsum", bufs=2, space="PSUM") as apsum, \
             tc.tile_pool(name="esb", bufs=3) as esb, \
             tc.tile_pool(name="nsb", bufs=2) as nsb:
            for m in range(2):          # head-pair == m-tile == o-conv kgroup
                for qi in range(NROW):
                    qsl = slice(qi * NT, (qi + 1) * NT)
                    av = [apsum.tile([128, NT], F32, tag="avps", name="avps") for _ in range(2)]
                    for grp in range(NGRP):
                        for hh in range(2):
                            h = 2 * m + hh
                            sp = spsum.tile([128, KJG, 512], F32, tag="sps", name="sps")
                            for j in range(KJG):
                                kjt = grp * KJG + j
                                lhsT = k_sb[m][64 * hh:64 * hh + 64, kjt * 128:(kjt + 1) * 128]
                                rhs = q_sb[m][64 * hh:64 * hh + 64, qsl]
                                nc.tensor.matmul(
                                    sp[:, j, 0:NT], lhsT, rhs,
                                    start=True, stop=True,
                                    tile_position=(64 * hh, 0),
                                )
                            et = esb.tile([128, KJG, NT], BF, tag="et", name="et")
                            nc.scalar.activation(et[:], sp[:, :, 0:NT], EXP, scale=0.125)
                            for j in range(KJG):
                                kjt = grp * KJG + j
                                lhsT = vt_sb[h][:, kjt, 0:65]
                                nc.tensor.matmul(
                                    av[hh][0:65, :], lhsT, et[:, j, :],
                                    start=(kjt == 0), stop=(kjt == NKJ - 1),
                                )
                    for hh in range(2):
                        # evict AV psum to SBUF right away to recycle the bank
                        avf = nsb.tile([128, NT], F32, tag="avf", name="avf")
                        nc.vector.tensor_copy(avf[0:65, :], av[hh][0:65, :])
                        # den sits in row 64; DMA remaps partition 64 -> 0
                        dn = nsb.tile([1, NT], F32, tag="dn", name="dn")
                        nc.sync.dma_start(dn[:], avf[64:65, :])
                        rc = nsb.tile([1, NT], F32, tag="rc", name="rc")
                        nc.vector.reciprocal_approx_fast(rc[:], dn[:])
                        rb = nsb.tile([64, NT], F32, tag="rb", name="rb")
                        nc.gpsimd.partition_broadcast(rb[:], rc[:])
                        tmp = nsb.tile([64, NT], BF, tag="tmp", name="tmp")
                        nc.vector.tensor_mul(tmp[:], avf[0:64, :], rb[:])
                        dst = opad[m][64 * hh:64 * hh + 64, qi * RT + 1: qi * RT + RT + 1, 1:49]
                        nc.sync.dma_start(dst, tmp[:].rearrange("p (r c) -> p r c", c=W))

        # ---------------- phase 3: o-conv partial ----------------
        with tc.tile_pool(name="opsum", bufs=6, space="PSUM") as opsum, \
             tc.tile_pool(name="osb", bufs=3) as osb:
            for mo in range(2):
                ps = [opsum.tile([128, NT], F32, tag="ops", name="ops") for _ in range(NROW)]
                first = True
                for kg in range(2):
                    for t in range(9):
                        ky, kx = t // 3, t % 3
                        lhsT = wo_sb[:, t, kg, mo * 128:(mo + 1) * 128]
                        last = (kg == 1 and t == 8)
                        for r in range(NROW):
                            rhs = opad[kg][:, r * RT + ky: r * RT + ky + RT, kx: kx + W]
                            nc.tensor.matmul(ps[r][:], lhsT, rhs, start=first, stop=last)
                        first = False
                for r in range(NROW):
                    ot = osb.tile([128, NT], F32, tag="osb", name="osb")
                    nc.vector.tensor_copy(ot[:], ps[r][:])
                    nc.sync.dma_start(out_d[mo, :, r * NT:(r + 1) * NT], ot[:])

    nc.compile()
    _cached = nc
    return nc


def make_in_maps(hidden_states, wq, wk, wv, wo):
    """Shard + pre-transform full inputs into 8 per-core input dicts."""
    bf = ml_dtypes.bfloat16
    hidden_states = np.asarray(hidden_states, np.float32)
    in_maps = []
    for core in range(NCORES):
        b, g = core // 2, core % 2
        xp = np.zeros((C, 50, 50), np.float32)
        xp[:, 1:49, 1:49] = hidden_states[b]
        xpad = np.ascontiguousarray(xp.reshape(2, 128, 50, 50)).astype(bf)
        wstk = np.stack(
            [
                np.asarray(w, np.float32)[g * 256:(g + 1) * 256]
                .transpose(2, 3, 1, 0)
                .reshape(9, 2, 128, 256)
                for w in (wq, wk, wv)
            ]
        ).astype(bf)
        wog = (
            np.asarray(wo, np.float32)[:, g * 256:(g + 1) * 256]
            .transpose(2, 3, 1, 0)
            .reshape(9, 2, 128, 256)
            .astype(bf)
        )
        in_maps.append({"xpad": xpad, "wqkv": wstk, "wo": wog})
    return in_maps


def combine_outputs(per_core_outs):
    """Sum the two head-group partials per batch sample."""
    out = np.empty((B, C, H, W), np.float32)
    for b in range(B):
        acc = per_core_outs[2 * b].reshape(C, HW).astype(np.float32) + \
              per_core_outs[2 * b + 1].reshape(C, HW).astype(np.float32)
        out[b] = acc.reshape(C, H, W)
    return out


def kernel(hidden_states, wq, wk, wv, wo):
    from concourse.bass_utils import run_bass_kernel_spmd

    nc = _build()
    in_maps = make_in_maps(hidden_states, wq, wk, wv, wo)
    res = run_bass_kernel_spmd(nc, in_maps, core_ids=list(range(NCORES)))
    return combine_outputs([r["out"] for r in res.results])
